# revision 6
# baseline (speedup 1.0000x reference)
"""Trainium2 Bass kernel for the jet autoencoder (dense_cnn).

Strategy: pure data-parallel over 8 NeuronCores (8192 events each = exactly
8 ghost batches of 1024 events). Per core, the conv chain runs with channels
on partitions and (event, pixel) positions on the free axis, one ghost batch
(4096 positions) at a time:

  conv1x1    -> TensorE matmuls into PSUM (two 4-bank half tiles per layer)
  gbn stats  -> VectorE bn_stats/bn_aggr on the PSUM pre-activations
  gbn+silu   -> one ScalarE activation pass (per-partition scale/bias) PSUM->SBUF
  residuals  -> folded into the next conv as a second accumulated matmul

Conv biases are dropped: gbn subtracts the per-ghost-batch mean, so any
per-channel bias added before the norm cancels exactly.

The pt/eta/phi/m physics (jPxPyPzE) runs once per core on dense
(128 x 256) tiles; the reconstruction physics runs once on the collected
(96 x 1024) repacked encoder output. Outputs are written channel-major
(C, 32768) and untangled on the host.
"""

import sys

if "/opt/trn_rl_repo" not in sys.path:
    sys.path.insert(0, "/opt/trn_rl_repo")

import numpy as np

B = 65536
NCORES = 8
BS = B // NCORES          # events per core
GBE = 1024                # events per ghost batch
NGBL = BS // GBE          # ghost batches per core
PIX = 4
POS = GBE * PIX           # positions per ghost batch
NPOS = BS * PIX           # positions per core
D = 128
DB = 16
EPS = 1e-5

_CACHE = {}


def _build_module():
    from contextlib import ExitStack

    import concourse.tile as tile
    from concourse import bacc, mybir

    dt = mybir.dt.float32
    AF = mybir.ActivationFunctionType
    ALU = mybir.AluOpType

    nc = bacc.Bacc(trn_type="TRN2")

    j = nc.dram_tensor("j", [BS, 4, PIX], dt, kind="ExternalInput")
    din = {}
    for nm, shp in [
        ("wie", [3, D]), ("wec", [D, D]), ("wbi", [D, DB]),
        ("wbo", [DB, D]), ("wdc", [D, D]), ("wor", [D, 3]),
        ("gie", [D, 1]), ("bie", [D, 1]), ("gec", [D, 1]), ("bec", [D, 1]),
        ("gbi", [DB, 1]), ("bbi", [DB, 1]), ("gbo", [D, 1]), ("bbo", [D, 1]),
        ("gor", [3, 1]), ("bor", [3, 1]),
    ]:
        din[nm] = nc.dram_tensor(nm, shp, dt, kind="ExternalInput")
    o_jpx = nc.dram_tensor("o_jpx", [4, NPOS], dt, kind="ExternalOutput")
    o_rec4 = nc.dram_tensor("o_rec4", [4, NPOS], dt, kind="ExternalOutput")
    o_rec3 = nc.dram_tensor("o_rec3", [3, NPOS], dt, kind="ExternalOutput")
    o_z = nc.dram_tensor("o_z", [DB, NPOS], dt, kind="ExternalOutput")

    with tile.TileContext(nc) as tc, ExitStack() as ctx:
        sing = ctx.enter_context(tc.tile_pool(name="sing", bufs=1))
        dense = ctx.enter_context(tc.tile_pool(name="dense", bufs=1))
        jpp = ctx.enter_context(tc.tile_pool(name="jpp", bufs=1))
        xa = ctx.enter_context(tc.tile_pool(name="xa", bufs=2))
        xs = ctx.enter_context(tc.tile_pool(name="xs", bufs=2))
        zp = ctx.enter_context(tc.tile_pool(name="zp", bufs=2))
        orp = ctx.enter_context(tc.tile_pool(name="orp", bufs=1))
        stp = ctx.enter_context(tc.tile_pool(name="stp", bufs=2))
        smp = ctx.enter_context(tc.tile_pool(name="smp", bufs=2))
        recp = ctx.enter_context(tc.tile_pool(name="recp", bufs=1))
        psp = ctx.enter_context(tc.tile_pool(name="psp", bufs=2, space="PSUM"))

        # ---- constants / weights -------------------------------------------
        w_t = {}
        for nm in ["wie", "wec", "wbi", "wbo", "wdc", "wor"]:
            w_t[nm] = sing.tile(list(din[nm].shape), dt, tag=nm, name=nm)
            nc.sync.dma_start(out=w_t[nm], in_=din[nm].ap())
        nm_t = {}
        for nm in ["gie", "bie", "gec", "bec", "gbi", "bbi", "gbo", "bbo",
                   "gor", "bor"]:
            nm_t[nm] = sing.tile(list(din[nm].shape), dt, tag=nm, name=nm)
            nc.sync.dma_start(out=nm_t[nm], in_=din[nm].ap())
        eps_t = sing.tile([128, 1], dt, tag="eps", name="eps")
        nc.vector.memset(eps_t, EPS)
        halfpi_t = sing.tile([128, 1], dt, tag="halfpi", name="halfpi")
        nc.vector.memset(halfpi_t, float(np.pi / 2))

        # ---- load j contiguously: partition = event//64 --------------------
        jr = sing.tile([128, 64, 4, PIX], dt, tag="jr")
        nc.sync.dma_start(out=jr, in_=j.ap().rearrange("(h l) c p -> h l c p", l=64))

        pt = jr[:, :, 0, :]
        eta = jr[:, :, 1, :]
        phi = jr[:, :, 2, :]
        m = jr[:, :, 3, :]

        # dense channel tiles used to assemble per-gb conv input rows
        lp_d = sing.tile([128, 64, PIX], dt, tag="lp_d")
        nc.scalar.activation(out=lp_d, in_=pt, func=AF.Ln, bias=1.0, scale=1.0)
        eta_d = sing.tile([128, 64, PIX], dt, tag="eta_d")
        nc.vector.tensor_copy(out=eta_d, in_=eta)
        phi_d = sing.tile([128, 64, PIX], dt, tag="phi_d")
        nc.vector.tensor_copy(out=phi_d, in_=phi)

        # ---- dense physics: jPxPyPzE ---------------------------------------
        def dtile(tag):
            return dense.tile([128, 64, PIX], dt, tag=tag, name=tag)

        TWOPI = float(2 * np.pi)
        HPI = float(np.pi / 2)
        cph = dtile("cph")
        nc.vector.tensor_scalar(cph, phi, HPI, -TWOPI, ALU.is_gt, ALU.mult)
        nc.vector.tensor_add(cph, cph, phi)
        nc.scalar.activation(out=cph, in_=cph, func=AF.Sin, bias=halfpi_t, scale=1.0)
        sph = dtile("sph")
        nc.scalar.activation(out=sph, in_=phi, func=AF.Sin, bias=0.0, scale=1.0)
        ee = dtile("ee")
        nc.scalar.activation(out=ee, in_=eta, func=AF.Exp, bias=0.0, scale=1.0)
        eem = dtile("eem")
        nc.scalar.activation(out=eem, in_=eta, func=AF.Exp, bias=0.0, scale=-1.0)

        px = dtile("px")
        nc.vector.tensor_mul(px, pt, cph)
        py = dtile("py")
        nc.vector.tensor_mul(py, pt, sph)
        pth = dtile("pth")
        nc.vector.tensor_scalar_mul(pth, pt, 0.5)
        shd = dtile("shd")
        nc.vector.tensor_sub(shd, ee, eem)
        pz = dtile("pz")
        nc.vector.tensor_mul(pz, pth, shd)
        chd = dtile("chd")
        nc.vector.tensor_add(chd, ee, eem)
        u = dtile("u")
        nc.vector.tensor_mul(u, pth, chd)
        nc.vector.tensor_mul(u, u, u)
        m2 = dtile("m2")
        nc.vector.tensor_mul(m2, m, m)
        nc.vector.tensor_add(u, u, m2)
        en = dtile("en")
        nc.scalar.activation(out=en, in_=u, func=AF.Sqrt, bias=0.0, scale=1.0)

        for ci, src in enumerate([px, py, pz, en]):
            nc.sync.dma_start(
                out=o_jpx.ap()[ci].rearrange("(h f) -> h f", h=128),
                in_=src.rearrange("h l p -> h (l p)"),
            )

        # ---- collected encoder output for the rec phase --------------------
        or_all = sing.tile([96, 1024], dt, tag="or_all")

        # ---- per-ghost-batch conv chain ------------------------------------
        def conv_gbn_silu(w_tile, rhs_list, dout, g_tile, be_tile, out_tile):
            """out = silu(gbn(W @ sum(rhs_list))) over this ghost batch."""
            halves = []
            for h in range(2):
                pst = psp.tile([128, 4, 512], dt, tag="ps")
                for c in range(4):
                    off = h * 2048 + c * 512
                    for ri, rhs in enumerate(rhs_list):
                        nc.tensor.matmul(
                            pst[0:dout, c, :], w_tile, rhs[:, off:off + 512],
                            start=(ri == 0), stop=(ri == len(rhs_list) - 1),
                        )
                halves.append(pst)
            stats = stp.tile([128, 8, 6], dt, tag="stats")
            for h, pst in enumerate(halves):
                for c in range(4):
                    nc.vector.bn_stats(out=stats[0:dout, h * 4 + c, :],
                                       in_=pst[0:dout, c, :])
            mv = smp.tile([128, 2], dt, tag="mv")
            nc.vector.bn_aggr(out=mv[0:dout], in_=stats[0:dout])
            rstd = smp.tile([128, 1], dt, tag="rstd")
            nc.scalar.activation(out=rstd[0:dout], in_=mv[0:dout, 1:2],
                                 func=AF.Sqrt, bias=eps_t[0:dout], scale=1.0)
            nc.vector.reciprocal(out=rstd[0:dout], in_=rstd[0:dout])
            aa = smp.tile([128, 1], dt, tag="aa")
            nc.vector.tensor_mul(aa[0:dout], g_tile, rstd[0:dout])
            cb = smp.tile([128, 1], dt, tag="cb")
            nc.vector.tensor_mul(cb[0:dout], mv[0:dout, 0:1], aa[0:dout])
            nc.vector.tensor_sub(cb[0:dout], be_tile, cb[0:dout])
            for h, pst in enumerate(halves):
                nc.scalar.activation(
                    out=out_tile[0:dout, h * 2048:(h + 1) * 2048],
                    in_=pst[0:dout].rearrange("p a b -> p (a b)"),
                    func=AF.Silu, bias=cb[0:dout], scale=aa[0:dout],
                )
            return out_tile

        for g in range(NGBL):
            jp = jpp.tile([3, 16, 256], dt, tag="jp")
            for ci, src in enumerate([lp_d, eta_d, phi_d]):
                nc.sync.dma_start(
                    out=jp[ci:ci + 1],
                    in_=src[g * 16:(g + 1) * 16].rearrange("h l p -> h (l p)"),
                )
            jpf = jp.rearrange("c a b -> c (a b)")

            x1 = xa.tile([128, POS], dt, tag="xa")
            conv_gbn_silu(w_t["wie"], [jpf], D, nm_t["gie"], nm_t["bie"], x1)
            s2 = xs.tile([128, POS], dt, tag="xs")
            conv_gbn_silu(w_t["wec"], [x1], D, nm_t["gec"], nm_t["bec"], s2)
            z = zp.tile([DB, POS], dt, tag="z")
            conv_gbn_silu(w_t["wbi"], [x1, s2], DB, nm_t["gbi"], nm_t["bbi"], z)
            nc.sync.dma_start(out=o_z.ap()[:, g * POS:(g + 1) * POS], in_=z)
            x4 = xa.tile([128, POS], dt, tag="xa")
            conv_gbn_silu(w_t["wbo"], [z], D, nm_t["gbo"], nm_t["bbo"], x4)
            s5 = xs.tile([128, POS], dt, tag="xs")
            conv_gbn_silu(w_t["wdc"], [x4], D, nm_t["gec"], nm_t["bec"], s5)
            x6 = orp.tile([3, POS], dt, tag="or")
            conv_gbn_silu(w_t["wor"], [x4, s5], 3, nm_t["gor"], nm_t["bor"], x6)
            # repack (3, 4096) -> or_all[(c*32 + 4g) : +4, 0:1024]
            for c in range(3):
                nc.sync.dma_start(
                    out=or_all[c * 32 + 4 * g: c * 32 + 4 * g + 4],
                    in_=x6[c:c + 1].rearrange("o (q f) -> o q f", q=4),
                )

        # ---- rec physics on (32, 1024) channel slices ----------------------
        xo0 = or_all[0:32]
        xo1 = or_all[32:64]
        xo2 = or_all[64:96]

        def rtile(tag):
            return recp.tile([32, 1024], dt, tag=tag, name=tag)

        ea = rtile("ea")
        nc.scalar.activation(out=ea, in_=xo0, func=AF.Exp, bias=0.0, scale=1.0)
        eb = rtile("eb")
        nc.scalar.activation(out=eb, in_=xo0, func=AF.Exp, bias=0.0, scale=-1.0)
        rpt = rtile("rpt")
        nc.vector.tensor_add(rpt, ea, eb)
        nc.vector.tensor_scalar(rpt, rpt, 0.5, 39.0, ALU.mult, ALU.add)
        xc2 = rtile("xc2")
        nc.vector.tensor_copy(out=xc2, in_=xo2)
        cp2 = rtile("cp2")
        nc.vector.tensor_scalar(cp2, xc2, HPI, -TWOPI, ALU.is_gt, ALU.mult)
        nc.vector.tensor_add(cp2, cp2, xc2)
        nc.scalar.activation(out=cp2, in_=cp2, func=AF.Sin, bias=halfpi_t[0:32], scale=1.0)
        sp2 = rtile("sp2")
        nc.vector.tensor_scalar(sp2, xc2, float(np.pi), -TWOPI, ALU.is_gt, ALU.mult)
        nc.vector.tensor_add(sp2, sp2, xc2)
        nc.scalar.activation(out=sp2, in_=sp2, func=AF.Sin, bias=0.0, scale=1.0)
        rpx = rtile("rpx")
        nc.vector.tensor_mul(rpx, rpt, cp2)
        rpy = rtile("rpy")
        nc.vector.tensor_mul(rpy, rpt, sp2)
        nc.scalar.activation(out=ea, in_=xo1, func=AF.Exp, bias=0.0, scale=1.0)
        nc.scalar.activation(out=eb, in_=xo1, func=AF.Exp, bias=0.0, scale=-1.0)
        rpz = rtile("rpz")
        nc.vector.tensor_sub(rpz, ea, eb)
        nc.vector.tensor_scalar_mul(rpz, rpz, 0.5)
        nc.vector.tensor_mul(rpz, rpt, rpz)
        q1 = rtile("q1")
        nc.vector.tensor_mul(q1, rpt, rpt)
        q2 = rtile("q2")
        nc.vector.tensor_mul(q2, rpz, rpz)
        nc.vector.tensor_add(q1, q1, q2)
        ren = rtile("ren")
        nc.scalar.activation(out=ren, in_=q1, func=AF.Sqrt, bias=0.0, scale=1.0)

        for ci, src in enumerate([rpt, xo1, xo2]):
            nc.sync.dma_start(
                out=o_rec3.ap()[ci].rearrange("(q f) -> q f", q=32), in_=src)
        for ci, src in enumerate([rpx, rpy, rpz, ren]):
            nc.sync.dma_start(
                out=o_rec4.ap()[ci].rearrange("(q f) -> q f", q=32), in_=src)

    nc.compile()
    return nc


def _get_module():
    if "nc" not in _CACHE:
        _CACHE["nc"] = _build_module()
    return _CACHE["nc"]


def kernel(**inputs):
    from concourse.bass_utils import run_bass_kernel_spmd

    j = np.ascontiguousarray(np.asarray(inputs["j"], dtype=np.float32))
    f32 = lambda x: np.ascontiguousarray(np.asarray(x, dtype=np.float32))
    col = lambda x: f32(x).reshape(-1, 1)

    shared = {
        "wie": f32(inputs["w_ie"]).T.copy(),
        "wec": f32(inputs["w_ec"]).T.copy(),
        "wbi": f32(inputs["w_bi"]).T.copy(),
        "wbo": f32(inputs["w_bo"]).T.copy(),
        "wdc": f32(inputs["w_dc"]).T.copy(),
        "wor": f32(inputs["w_or"]).T.copy(),
        "gie": col(inputs["g_ie"]), "bie": col(inputs["be_ie"]),
        "gec": col(inputs["g_ec"]), "bec": col(inputs["be_ec"]),
        "gbi": col(inputs["g_bi"]), "bbi": col(inputs["be_bi"]),
        "gbo": col(inputs["g_bo"]), "bbo": col(inputs["be_bo"]),
        "gor": col(inputs["g_or"]), "bor": col(inputs["be_or"]),
    }

    nc = _get_module()
    shards = j.reshape(NCORES, BS, 4, PIX)
    in_maps = [{"j": np.ascontiguousarray(shards[c]), **shared}
               for c in range(NCORES)]
    res = run_bass_kernel_spmd(nc, in_maps, core_ids=list(range(NCORES)))

    def chan_major(name, c, ch):
        return res.results[c][name].reshape(ch, BS, PIX).transpose(1, 0, 2)

    jpx = np.concatenate([chan_major("o_jpx", c, 4) for c in range(NCORES)])
    rec4 = np.concatenate([chan_major("o_rec4", c, 4) for c in range(NCORES)])
    rec3 = np.concatenate([chan_major("o_rec3", c, 3) for c in range(NCORES)])
    z = np.concatenate([chan_major("o_z", c, DB) for c in range(NCORES)])
    return (jpx, rec4, j, rec3, z)
